# revision 1
# baseline (speedup 1.0000x reference)
"""Multi-head attention (B=4, N=2048, C=1024, H=16, HD=64) on 8 TRN2 NeuronCores.

Self-contained: takes the FULL unsharded inputs of the reference problem,
shards across 8 cores, runs a Bass/Tile kernel SPMD, and reassembles the
full output.

Sharding (tensor-parallel over heads x data-parallel over batch):
  core i -> batch b = i//2, head-group g = i%2 (8 of the 16 heads).
  Each core computes its heads' QKV projections, full-sequence attention,
  and a partial output projection (its heads' contribution to all output
  channels). Unsharding sums the two partial projections per batch — the
  reduction that a TP all-reduce would do, done at gather time.

Host-side prep transposes x and the weight slices so that on-device every
matmul contracts over the SBUF partition axis with natural layouts (no
on-device transposes of x or weights are needed).

Per-core pipeline (bf16 matmuls, f32 PSUM accumulation):
  KT [DH,N] = wkT.T @ xT ;  QT [DH,N] = wqT.T @ xT          (d on partitions)
  V  [N,DH] = xT.T @ wvT  (+ ones column per head -> softmax denominators)
  per head, per 1024-query block:
    S^T [keys,q] = KT_h @ QT_h        (contraction = head dim 64)
    E^T = exp(S^T / sqrt(64))         (ScalarE, PSUM -> SBUF bf16;
                                       no max-subtraction: scores ~ N(0,1))
    O [q,65] = sum_k E^T_k.T @ Vaug_k (ones column accumulates the denom)
    O_h = O[:,:64] * (1/O[:,64]) ; PE-transpose -> OT [DH,N]
  out [N,OUTC] = OT.T @ wpT + pb      (partial: this head-group's channels)
"""

import sys

sys.path.insert(0, "/opt/trn_rl_repo")

from contextlib import ExitStack

import numpy as np

import concourse.bass as bass
import concourse.tile as tile
from concourse import mybir
from concourse.masks import make_identity
from concourse.bass_utils import run_bass_kernel_spmd

F32 = mybir.dt.float32
BF16 = mybir.dt.bfloat16

B, N, C, H = 4, 2048, 1024, 16
HH = H // 2          # heads per core
HD = C // H          # head dim
DH = HH * HD         # attention channels per core
OUTC = C


def _split_multi_waits(nc, max_waits=1):
    """The pinned walrus build rejects >1 SyncWait on engine CTRL
    instructions; move extra waits onto preceding NOPs."""
    n_split = 0
    for bb in nc.main_func.blocks:
        insts = list(bb.instructions)
        new_insts = []
        changed = False
        for ins in insts:
            si = getattr(ins, "sync_info", None)
            nm = type(ins).__name__
            is_dma = "Dma" in nm or "TensorLoad" in nm or "TensorSave" in nm
            if si is not None and not is_dma:
                waits = list(si.on_wait)
                if len(waits) > max_waits:
                    head, tail = waits[:-max_waits], waits[-max_waits:]
                    for i in range(0, len(head), max_waits):
                        new_insts.append(
                            mybir.InstNoOp(
                                name=f"{ins.name}-ws{n_split}-{i}",
                                engine=ins.engine,
                                sync_info=mybir.SyncInfo(
                                    on_wait=head[i : i + max_waits], on_update=[]
                                ),
                                bass_nofuse=True,
                            )
                        )
                    ins.sync_info = mybir.SyncInfo(
                        on_wait=tail, on_update=list(si.on_update)
                    )
                    n_split += 1
                    changed = True
            new_insts.append(ins)
        if changed:
            bb.instructions = new_insts
    return n_split


def build_attention_nc(R=1):
    CO = C // 128
    DO = DH // 128
    NO128 = N // 128
    NO512 = N // 512
    KO = N // 128
    QB = 1024
    NQB = N // QB
    QBO = QB // 128
    HPT = 128 // HD
    SCALE = float(HD) ** -0.5
    OUTF = 512
    OUTCO = OUTC // OUTF

    nc = bass.Bass()
    xT_e = nc.declare_dram_parameter("xT", [C, N], F32, isOutput=False)
    wqT_e = nc.declare_dram_parameter("wqT", [C, DH], F32, isOutput=False)
    wkT_e = nc.declare_dram_parameter("wkT", [C, DH], F32, isOutput=False)
    wvT_e = nc.declare_dram_parameter("wvT", [C, DH], F32, isOutput=False)
    wpT_e = nc.declare_dram_parameter("wpT", [DH, OUTC], F32, isOutput=False)
    pb_e = nc.declare_dram_parameter("pb", [1, OUTC], F32, isOutput=False)
    out_e = nc.declare_dram_parameter("out", [N, OUTC], F32, isOutput=True)

    with tile.TileContext(nc) as tc:

        def body(_iv=None):
            with ExitStack() as ctx:
                persist = ctx.enter_context(tc.tile_pool(name="persist", bufs=1))
                KT = persist.tile([128, DO, N], BF16)
                QT = persist.tile([128, DO, N], BF16)
                VN = persist.tile([128, NO128, HH, HD + 1], BF16)
                OT = persist.tile([128, DO, N], BF16)
                WPb = persist.tile([128, DO, OUTC], BF16)
                bias_sb = persist.tile([128, OUTC], F32)
                ident = persist.tile([128, 128], BF16)
                make_identity(nc, ident)

                # ---- load + cast + qkv projections ----
                with tc.tile_pool(name="load", bufs=8) as ld, tc.tile_pool(
                    name="wsb", bufs=1
                ) as wsb, tc.tile_pool(name="psA", bufs=4, space="PSUM") as psA:
                    XTb = wsb.tile([128, CO, N], BF16)
                    WQb = wsb.tile([128, CO, DH], BF16)
                    WKb = wsb.tile([128, CO, DH], BF16)
                    WVb = wsb.tile([128, CO, DH], BF16)

                    # PE warm-up burst: ~8us of dummy matmuls keeps the HAM
                    # clock gate at full rate through the DMA-paced load phase
                    wps = psA.tile([128, 512], F32, tag="psA")
                    for wi in range(40):
                        nc.tensor.matmul(
                            wps,
                            lhsT=XTb[:, 0, 0:128],
                            rhs=XTb[:, 0, 0:512],
                            start=(wi == 0),
                            stop=(wi == 39),
                        )
                    for co in range(CO):
                        st = ld.tile([128, N], F32, tag="stage")
                        nc.sync.dma_start(
                            out=st, in_=xT_e[co * 128 : (co + 1) * 128, :]
                        )
                        nc.vector.tensor_copy(XTb[:, co, :], st)
                    for w_e, Wb in ((wqT_e, WQb), (wkT_e, WKb), (wvT_e, WVb)):
                        for co in range(CO):
                            st = ld.tile([128, N], F32, tag="stage")
                            nc.sync.dma_start(
                                out=st[:, :DH], in_=w_e[co * 128 : (co + 1) * 128, :]
                            )
                            nc.vector.tensor_copy(Wb[:, co, :], st[:, :DH])
                    for do in range(DO):
                        st = ld.tile([128, N], F32, tag="stage")
                        nc.sync.dma_start(
                            out=st[:, :OUTC], in_=wpT_e[do * 128 : (do + 1) * 128, :]
                        )
                        nc.vector.tensor_copy(WPb[:, do, :], st[:, :OUTC])
                    pb_bcast = bass.AP(
                        tensor=pb_e.tensor if hasattr(pb_e, "tensor") else pb_e,
                        offset=0,
                        ap=[[0, 128], [1, OUTC]],
                    )
                    nc.sync.dma_start(out=bias_sb, in_=pb_bcast)

                    # KT / QT (do-outer so head pair 0 is ready first)
                    for do in range(DO):
                        for Wb, DST in ((WKb, KT), (WQb, QT)):
                            for no in range(NO512):
                                ps = psA.tile([128, 512], F32, tag="psA")
                                for co in range(CO):
                                    nc.tensor.matmul(
                                        ps,
                                        lhsT=Wb[:, co, do * 128 : (do + 1) * 128],
                                        rhs=XTb[:, co, no * 512 : (no + 1) * 512],
                                        start=(co == 0),
                                        stop=(co == CO - 1),
                                    )
                                nc.vector.tensor_copy(
                                    DST[:, do, no * 512 : (no + 1) * 512], ps
                                )
                    # V natural + ones column (-> softmax denominators)
                    nc.vector.memset(VN[:, :, :, HD], 1.0)
                    for no in range(NO128):
                        ps = psA.tile([128, DH], F32, tag="psV")
                        for co in range(CO):
                            nc.tensor.matmul(
                                ps,
                                lhsT=XTb[:, co, no * 128 : (no + 1) * 128],
                                rhs=WVb[:, co, :],
                                start=(co == 0),
                                stop=(co == CO - 1),
                            )
                        nc.vector.tensor_copy(
                            VN[:, no, :, 0:HD],
                            ps.rearrange("p (h d) -> p h d", h=HH),
                        )

                # ---- attention ----
                with tc.tile_pool(name="et", bufs=2) as etp, tc.tile_pool(
                    name="attn_sm", bufs=4
                ) as smp, tc.tile_pool(
                    name="psS", bufs=2, space="PSUM"
                ) as psS, tc.tile_pool(
                    name="psO", bufs=2, space="PSUM"
                ) as psO, tc.tile_pool(name="psT", bufs=2, space="PSUM") as psT:
                    for h in range(HH):
                        row = (h % HPT) * HD
                        dslot = h // HPT
                        for qb in range(NQB):
                            ET = etp.tile([128, KO, QB], BF16, tag="ET")
                            for ko in range(KO):
                                ps_s = psS.tile([128, QB], F32, tag="psS")
                                for q2 in range(QB // 512):
                                    nc.tensor.matmul(
                                        ps_s[:, q2 * 512 : (q2 + 1) * 512],
                                        lhsT=KT[
                                            row : row + HD,
                                            dslot,
                                            ko * 128 : (ko + 1) * 128,
                                        ],
                                        rhs=QT[
                                            row : row + HD,
                                            dslot,
                                            qb * QB
                                            + q2 * 512 : qb * QB
                                            + (q2 + 1) * 512,
                                        ],
                                        start=True,
                                        stop=True,
                                    )
                                nc.scalar.activation(
                                    out=ET[:, ko, :],
                                    in_=ps_s,
                                    func=mybir.ActivationFunctionType.Exp,
                                    scale=SCALE,
                                )
                            # PV batched; normalize/transpose deferred so the
                            # PE never waits on the DVE epilogue chain
                            Ostg = smp.tile([128, QBO, HD + 1], F32, tag="ostg")
                            for qo in range(QBO):
                                ps_o = psO.tile([128, HD + 1], F32, tag="psO")
                                for ko in range(KO):
                                    nc.tensor.matmul(
                                        ps_o,
                                        lhsT=ET[:, ko, qo * 128 : (qo + 1) * 128],
                                        rhs=VN[:, ko, h, :],
                                        start=(ko == 0),
                                        stop=(ko == KO - 1),
                                    )
                                nc.vector.tensor_copy(Ostg[:, qo, :], ps_o)
                            Rcp = smp.tile([128, QBO], F32, tag="rcp")
                            nc.vector.reciprocal(Rcp, Ostg[:, :, HD])
                            for qo in range(QBO):
                                ob = smp.tile([128, HD], BF16, tag="ob")
                                nc.vector.tensor_tensor(
                                    ob,
                                    Ostg[:, qo, 0:HD],
                                    Rcp[:, qo : qo + 1].to_broadcast([128, HD]),
                                    mybir.AluOpType.mult,
                                )
                                ps_t = psT.tile([HD, 128], BF16, tag="psT")
                                nc.tensor.transpose(ps_t, ob, ident)
                                nc.vector.tensor_copy(
                                    OT[
                                        row : row + HD,
                                        dslot,
                                        qb * QB + qo * 128 : qb * QB + (qo + 1) * 128,
                                    ],
                                    ps_t,
                                )

                # ---- output projection (+ bias) ----
                with tc.tile_pool(name="ostage", bufs=3) as osp, tc.tile_pool(
                    name="psC", bufs=4, space="PSUM"
                ) as psC:
                    for no in range(NO128):
                        st = osp.tile([128, OUTC], F32, tag="ostage")
                        for oc in range(OUTCO):
                            ps = psC.tile([128, OUTF], F32, tag="psC")
                            for ci in range(DO):
                                nc.tensor.matmul(
                                    ps,
                                    lhsT=OT[:, ci, no * 128 : (no + 1) * 128],
                                    rhs=WPb[:, ci, oc * OUTF : (oc + 1) * OUTF],
                                    start=(ci == 0),
                                    stop=(ci == DO - 1),
                                )
                            nc.vector.tensor_tensor(
                                st[:, oc * OUTF : (oc + 1) * OUTF],
                                ps,
                                bias_sb[:, oc * OUTF : (oc + 1) * OUTF],
                                mybir.AluOpType.add,
                            )
                        nc.sync.dma_start(
                            out=out_e[no * 128 : (no + 1) * 128, :], in_=st
                        )

        if R == 1:
            body()
        else:
            with tc.For_i(0, R, 1) as iv:
                body(iv)

    _split_multi_waits(nc)
    return nc


def shard_inputs(x, qkv_w, proj_w, proj_b):
    in_maps = []
    for i in range(8):
        b, g = i // 2, i % 2
        sl = slice(g * DH, (g + 1) * DH)
        xT = np.ascontiguousarray(x[b].T).astype(np.float32)
        wqT = np.ascontiguousarray(qkv_w[0 * C : 1 * C][sl, :].T).astype(np.float32)
        wkT = np.ascontiguousarray(qkv_w[1 * C : 2 * C][sl, :].T).astype(np.float32)
        wvT = np.ascontiguousarray(qkv_w[2 * C : 3 * C][sl, :].T).astype(np.float32)
        wpT = np.ascontiguousarray(proj_w[:, sl].T).astype(np.float32)
        pb = (proj_b if g == 0 else np.zeros_like(proj_b)).reshape(1, -1)
        in_maps.append(
            {
                "xT": xT,
                "wqT": wqT,
                "wkT": wkT,
                "wvT": wvT,
                "wpT": wpT,
                "pb": np.ascontiguousarray(pb).astype(np.float32),
            }
        )
    return in_maps


_CACHED_NC = None


def kernel(x, qkv_w, proj_w, proj_b):
    """Full inputs in, full output out. Shards over 8 NeuronCores."""
    global _CACHED_NC
    x = np.asarray(x, dtype=np.float32)
    qkv_w = np.asarray(qkv_w, dtype=np.float32)
    proj_w = np.asarray(proj_w, dtype=np.float32)
    proj_b = np.asarray(proj_b, dtype=np.float32)

    if _CACHED_NC is None:
        _CACHED_NC = build_attention_nc(R=1)
    nc = _CACHED_NC

    in_maps = shard_inputs(x, qkv_w, proj_w, proj_b)
    res = run_bass_kernel_spmd(nc, in_maps, core_ids=list(range(8)))
    out = np.empty((B, N, OUTC), dtype=np.float32)
    for b in range(B):
        out[b] = res.results[2 * b]["out"] + res.results[2 * b + 1]["out"]
    return out

